# revision 1
# baseline (speedup 1.0000x reference)
"""Trainium2 Bass kernel for nn_AttentionNet (audio-visual attention).

Data-parallel across 8 NeuronCores: batch B=256 split 32 per core, i.e.
320 (b,t) rows and 320*49 = 15680 visual rows per core.

Per-core math (n indexes the 320 rows, s in [0,49), d/e in [0,512)):
    a_t = relu(audio @ Wa.T + ba)            [N,512]
    v_t = relu(vis @ Wv.T + bv)              [N,49,512]
    a_s = a_t @ Aa.T                         [N,49]
    v_s = v_t @ Av.T                         [N,49,49]
    f   = (tanh(a_s[:,:,None] + v_s)) @ Af.T [N,49]
    att = softmax_s(f)
    out = att @ vis                          [N,512]

Layout: visual rows (n,s) are transposed on the PE into visT[d,(n,s)]
column blocks of 490 (10 n's), so the d/e contractions run as full-width
matmuls in float32r (1 cycle/row on the PE, fp32 storage).  The softmax
runs unnormalized on the [1, 490] f-row; 1/Z is folded into the final
output transpose as a per-partition scale.
"""

import numpy as np

try:
    import concourse.bass as bass
except ImportError:
    import sys as _sys
    for _p in ("/opt/trn_rl_repo", "/root/.axon_site/_ro/trn_rl_repo"):
        if _p not in _sys.path:
            _sys.path.insert(0, _p)
    import concourse.bass as bass
import concourse.mybir as mybir
import concourse.tile as tile
from concourse import bacc

F32 = mybir.dt.float32
F32R = mybir.dt.float32r
AX = mybir.AxisListType
ALU = mybir.AluOpType
AF = mybir.ActivationFunctionType

NCORES = 8
B, T, S, D, E, A = 256, 10, 49, 512, 512, 128
NB = 10              # n's per column block
CB = NB * S          # 490 columns per block


def _r(ap):
    return ap.bitcast(F32R)


def _tr(nc, out, in_, ident):
    nc.tensor.transpose(out.bitcast(ident.dtype), in_, ident)


def build_module(n_n):
    """Build the Bass module for one core handling n_n (b,t) rows."""
    assert n_n % NB == 0
    rows = n_n * S
    nblk = n_n // NB
    n_rt = (rows + 127) // 128           # 128-row visual tiles
    n_nt = (n_n + 127) // 128            # 128-row n tiles (a-path / epilogue)

    nc = bacc.Bacc("TRN2", debug=False)

    aud_d = nc.dram_tensor("audio", [n_n, A], F32R, kind="ExternalInput").ap()
    vis_d = nc.dram_tensor("visual", [rows, D], F32R, kind="ExternalInput").ap()
    wvt_d = nc.dram_tensor("WvT", [128, 4, E], F32R, kind="ExternalInput").ap()
    wat_d = nc.dram_tensor("WaT", [128, E], F32R, kind="ExternalInput").ap()
    aat_d = nc.dram_tensor("AaT", [128, 4, 64], F32R, kind="ExternalInput").ap()
    avt_d = nc.dram_tensor("AvT", [128, 4, S], F32R, kind="ExternalInput").ap()
    aft_d = nc.dram_tensor("AfT", [S, 1], F32R, kind="ExternalInput").ap()
    ba_d = nc.dram_tensor("ba_l", [128, 4], F32, kind="ExternalInput").ap()
    bv_d = nc.dram_tensor("bv_l", [128, 4], F32, kind="ExternalInput").ap()
    idn_d = nc.dram_tensor("ident", [128, 128], F32R, kind="ExternalInput").ap()
    idf_d = nc.dram_tensor("identf", [128, 128], F32, kind="ExternalInput").ap()
    one_d = nc.dram_tensor("ones", [1, 128], F32R, kind="ExternalInput").ap()
    out_d = nc.dram_tensor("out", [n_n, D], F32, kind="ExternalOutput").ap()

    with tile.TileContext(nc) as tc, \
         tc.tile_pool(name="consts", bufs=1) as cp, \
         tc.tile_pool(name="vload", bufs=4) as vp, \
         tc.tile_pool(name="visT", bufs=6) as vtp, \
         tc.tile_pool(name="work", bufs=3) as wp, \
         tc.tile_pool(name="t7p", bufs=2) as t7p, \
         tc.tile_pool(name="dram", bufs=1, space="DRAM") as dp, \
         tc.tile_pool(name="ps_tr", bufs=2, space="PSUM") as ptr, \
         tc.tile_pool(name="ps_mm", bufs=3, space="PSUM") as pmm, \
         tc.tile_pool(name="ps_vs", bufs=2, space="PSUM") as pvs, \
         tc.tile_pool(name="ps_ft", bufs=1, space="PSUM") as pft:

        # ---------- constants ----------
        wvt = cp.tile([128, 4, E], F32R, tag="wvt")
        nc.sync.dma_start(wvt[:], wvt_d)
        wat = cp.tile([128, E], F32R, tag="wat")
        nc.sync.dma_start(wat[:], wat_d)
        aat = cp.tile([128, 4, 64], F32R, tag="aat")
        nc.sync.dma_start(aat[:], aat_d)
        avt = cp.tile([128, 4, S], F32R, tag="avt")
        nc.sync.dma_start(avt[:], avt_d)
        aft = cp.tile([S, 1], F32R, tag="aft")
        nc.sync.dma_start(aft[:], aft_d)
        ba = cp.tile([128, 4], F32, tag="ba")
        nc.sync.dma_start(ba[:], ba_d)
        bv = cp.tile([128, 4], F32, tag="bv")
        nc.sync.dma_start(bv[:], bv_d)
        idn = cp.tile([128, 128], F32R, tag="idn")
        nc.sync.dma_start(idn[:], idn_d)
        idf = cp.tile([128, 128], F32, tag="idf")
        nc.sync.dma_start(idf[:], idf_d)
        ones = cp.tile([1, 128], F32R, tag="ones")
        nc.sync.dma_start(ones[:], one_d)

        audT = cp.tile([128, n_n], F32R, tag="audT")     # audio.T  [a, n]
        atT = cp.tile([128, 4, n_n], F32R, tag="atT")    # a_t.T    [e, n]
        asr = cp.tile([1, rows], F32R, tag="asr")        # a_s row  [(n,s)]
        rinv = cp.tile([1, n_n], F32, tag="rinv")       # 1/Z per n
        outT = cp.tile([128, 4, n_n], F32, tag="outT")  # out.T    [d, n]

        # ---------- a-path prologue ----------
        for it in range(n_nt):
            n0 = it * 128
            nr = min(128, n_n - n0)
            an = wp.tile([128, A], F32R, tag="an")
            nc.sync.dma_start(an[:nr, :], aud_d[n0:n0 + nr, :])
            ps = ptr.tile([128, 128], F32, tag="tr")
            _tr(nc, ps[:, :nr], an[:nr, :], idn[:nr, :nr])
            nc.scalar.copy(audT[:, n0:n0 + nr], ps[:, :nr])

        for eo in range(4):
            ps = pmm.tile([128, max(CB, n_n)], F32, tag="mm")
            nc.tensor.matmul(ps[:, :n_n], wat[:, eo * 128:(eo + 1) * 128],
                             audT[:], start=True, stop=True)
            nc.scalar.activation(atT[:, eo, :], ps[:, :n_n], AF.Relu,
                                 bias=ba[:, eo:eo + 1])

        as_dram = dp.tile([1, rows], F32R, tag="asd")
        for it in range(n_nt):
            n0 = it * 128
            nr = min(128, n_n - n0)
            psa = pvs.tile([128, CB], F32, tag="vs")
            for eo in range(4):
                nc.tensor.matmul(psa[:nr, :64], atT[:, eo, n0:n0 + nr],
                                 aat[:, eo, :],
                                 start=(eo == 0), stop=(eo == 3))
            asn = wp.tile([128, S], F32R, tag="asn")
            nc.scalar.copy(asn[:nr, :], psa[:nr, :S])
            dst = as_dram[0:1, n0 * S:(n0 + nr) * S]
            nc.sync.dma_start(dst.rearrange("one (n s) -> (one n) s", s=S),
                              asn[:nr, :])
        nc.sync.dma_start(asr[:], as_dram[:])

        # ---------- main loop: visual transpose + per-block pipeline ----------
        visT = {}

        def get_visT(b):
            if b not in visT:
                visT[b] = vtp.tile([128, 4, CB], F32R, tag="visT",
                                   name=f"visT{b}")
            return visT[b]

        def do_rtile(t):
            r0 = t * 128
            rt = min(128, rows - r0)
            vn = vp.tile([128, D], F32R, tag="vn")
            nc.sync.dma_start(vn[:rt, :], vis_d[r0:r0 + rt, :])
            b0, b1 = r0 // CB, (r0 + rt - 1) // CB
            ps = ptr.tile([128, 4, 128], F32, tag="tr")
            for do in range(4):
                _tr(nc, ps[:, do, :rt], vn[:rt, do * 128:(do + 1) * 128],
                    idn[:rt, :rt])
            cp_op = nc.scalar.copy if t % 2 == 0 else nc.vector.tensor_copy
            for bb in range(b0, b1 + 1):
                lo = max(r0, bb * CB)
                hi = min(r0 + rt, (bb + 1) * CB)
                cp_op(get_visT(bb)[:, :, lo - bb * CB:hi - bb * CB],
                      ps[:, :, lo - r0:hi - r0])

        t_next = 0
        for b in range(nblk):
            t_end = (b * CB + CB - 1) // 128
            while t_next <= t_end and t_next < n_rt:
                do_rtile(t_next)
                t_next += 1
            vb = get_visT(b)

            # v_t.T = relu(Wv @ vis.T + bv)   [e, col]
            vt = wp.tile([128, 4, CB], F32R, tag="vtT")
            for eo in range(4):
                ps = pmm.tile([128, max(CB, n_n)], F32, tag="mm")
                for do in range(4):
                    nc.tensor.matmul(ps[:, :CB],
                                     wvt[:, do, eo * 128:(eo + 1) * 128],
                                     vb[:, do, :],
                                     start=(do == 0), stop=(do == 3))
                nc.scalar.activation(vt[:, eo, :], ps[:, :CB], AF.Relu,
                                     bias=bv[:, eo:eo + 1])

            # v_s.T + a_s  [f, col]
            psv = pvs.tile([128, CB], F32, tag="vs")
            for eo in range(4):
                nc.tensor.matmul(psv[:S, :], avt[:, eo, :],
                                 vt[:, eo, :], start=(eo == 0), stop=False)
            nc.tensor.matmul(psv[:S, :], ones[0:1, 0:S],
                             asr[0:1, b * CB:(b + 1) * CB],
                             start=False, stop=True)

            th = wp.tile([S, CB], F32R, tag="tanh")
            nc.scalar.activation(th[:], psv[:S, :], AF.Tanh)

            # f row = Af @ tanh  [1, col]
            psf = pft.tile([1, CB], F32, tag="ft")
            nc.tensor.matmul(psf[:], aft[:], th[:], start=True, stop=True)

            # unnormalized softmax: e = exp(f); Z per n; att = e (scaled later)
            ex = wp.tile([1, CB], F32R, tag="exp")
            nc.scalar.activation(ex[:], psf[:], AF.Exp)
            sm = wp.tile([1, NB], F32, tag="ssum")
            nc.vector.reduce_sum(sm[:], ex[:].bitcast(F32).rearrange("p (n s) -> p n s", n=NB),
                                 axis=AX.X)
            nc.vector.reciprocal(rinv[0:1, b * NB:(b + 1) * NB], sm[:])

            # broadcast e across 128 partitions via ones-matmul
            psb = pmm.tile([128, max(CB, n_n)], F32, tag="mm")
            nc.tensor.matmul(psb[:, :CB], ones[0:1, :], ex[:],
                             start=True, stop=True)
            ab = wp.tile([128, CB], F32, tag="attb")
            nc.scalar.copy(ab[:], psb[:, :CB])

            # out.T[d, n] += sum_s visT[d,(n,s)] * e[(n,s)]
            t7 = t7p.tile([128, 4, CB], F32, tag="t7")
            for do in range(4):
                nc.vector.tensor_tensor(t7[:, do, :], vb[:, do, :].bitcast(F32),
                                        ab[:], ALU.mult)
            nc.vector.reduce_sum(
                outT[:, :, b * NB:(b + 1) * NB],
                t7[:].rearrange("p f (n s) -> p f n s", n=NB), axis=AX.X)
            del visT[b]

        # ---------- epilogue: transpose out.T back, scale by 1/Z, store ----------
        for it in range(n_nt):
            n0 = it * 128
            nr = min(128, n_n - n0)
            psr = ptr.tile([128, 128], F32, tag="tr")
            _tr(nc, psr[:nr, 0:1], rinv[0:1, n0:n0 + nr], idf[0:1, 0:1])
            rin = wp.tile([128, 1], F32, tag="rin")
            nc.vector.tensor_copy(rin[:nr, :], psr[:nr, 0:1])
            on = wp.tile([128, D], F32, tag="on")
            for do in range(4):
                pso = ptr.tile([128, 128], F32, tag="tr")
                _tr(nc, pso[:nr, :], outT[:, do, n0:n0 + nr], idf[:, :])
                nc.scalar.activation(on[:nr, do * 128:(do + 1) * 128],
                                     pso[:nr, :], AF.Copy, scale=rin[:nr, 0:1])
            nc.sync.dma_start(out_d[n0:n0 + nr, :], on[:nr, :])

    nc.finalize()
    return nc


def prep_consts(Wa, ba, Wv, bv, Aa, Av, Af):
    f = np.float32
    c = {}
    c["WvT"] = np.ascontiguousarray(
        Wv.T.reshape(4, 128, E).transpose(1, 0, 2)).astype(f)
    c["WaT"] = np.ascontiguousarray(Wa.T).astype(f)
    aat = np.zeros((128, 4, 64), f)
    aat[:, :, :S] = Aa.T.reshape(4, 128, S).transpose(1, 0, 2)
    c["AaT"] = aat
    c["AvT"] = np.ascontiguousarray(
        Av.T.reshape(4, 128, S).transpose(1, 0, 2)).astype(f)
    c["AfT"] = np.ascontiguousarray(Af.reshape(1, S).T).astype(f)
    c["ba_l"] = np.ascontiguousarray(ba.reshape(4, 128).T).astype(f)
    c["bv_l"] = np.ascontiguousarray(bv.reshape(4, 128).T).astype(f)
    c["ident"] = np.eye(128, dtype=f)
    c["identf"] = np.eye(128, dtype=f)
    c["ones"] = np.ones((1, 128), dtype=f)
    return c


_CACHE = {}


def kernel(audio, visual, Wa, ba, Wv, bv, Aa, Av, Af):
    from concourse.bass_utils import run_bass_kernel_spmd

    audio = np.asarray(audio, np.float32)
    visual = np.asarray(visual, np.float32)
    n_n = (B // NCORES) * T  # 320

    if "nc" not in _CACHE:
        _CACHE["nc"] = build_module(n_n)
    nc = _CACHE["nc"]

    consts = prep_consts(np.asarray(Wa, np.float32), np.asarray(ba, np.float32),
                         np.asarray(Wv, np.float32), np.asarray(bv, np.float32),
                         np.asarray(Aa, np.float32), np.asarray(Av, np.float32),
                         np.asarray(Af, np.float32))
    bs = B // NCORES
    in_maps = []
    for c in range(NCORES):
        m = dict(consts)
        m["audio"] = np.ascontiguousarray(
            audio[c * bs:(c + 1) * bs].reshape(n_n, A))
        m["visual"] = np.ascontiguousarray(
            visual[c * bs:(c + 1) * bs].reshape(n_n * S, D))
        in_maps.append(m)

    res = run_bass_kernel_spmd(nc, in_maps, core_ids=list(range(NCORES)))
    _CACHE["last_res"] = res
    out = np.concatenate(
        [r["out"].reshape(bs, T, D) for r in res.results], axis=0)
    return out.astype(np.float32)



# revision 17
# speedup vs baseline: 1.0445x; 1.0445x over previous
"""Trainium2 Bass kernel for nn_AttentionNet (audio-visual attention).

Data-parallel across 8 NeuronCores: B=256 split 32/core -> 320 (b,t) rows
("n") and 15680 visual rows per core.

Per-core math (n in [0,320), s in [0,49), d/e in [0,512)):
    a_t = relu(audio @ Wa.T + ba)            [N,512]
    a_s = a_t @ Aa.T                         [N,49]
    v_t = relu(vis @ Wv.T + bv)              [N,49,512]
    v_s = v_t @ Av.T                         [N,49,49]
    f   = tanh(a_s[:,:,None] + v_s) @ Af.T   [N,49]
    att = softmax_s(f);  out = att @ vis     [N,512]

Implementation notes:
  * v_t / v_s run as fp8e4 DoubleRow matmuls (0.5 cyc/row, K=256/instr).
    Scales: visT holds 4*vis, wv8 holds 8*Wv -> psum = 32*pre-act;
    vt8 = relu(psum + 32*bv) = 32*v_t; av8 = 8*Av -> v_s psum = 256*v_s;
    a_s enters the same psum scaled by 256 (host-scaled Aa); tanh uses
    scale=1/256.
  * Work is chunked by 120/120/80 "n" rows (psum partition limit) with
    row-tiles of 120/120/112 and 490-column vblocks (49*10, n-aligned).
  * out = att @ vis runs on the PE against the *untransposed* vis tiles:
    per row-tile a masked attention matrix Att[row, n'] (e values scattered
    to each row's n-column) is built from a PE column-transpose of the exp
    row + a small mask multiply; one accumulating matmul per tile.
  * Softmax is unnormalized; 1/Z folds into the final psum->sbuf copy.
  * Elementwise work (psum copies / relu) rotates across Act, DVE and
    GpSimd(Pool) engines to keep all three below the PE/DMA roofline.
"""

import numpy as np

try:
    import concourse.bass as bass
except ImportError:
    import sys as _sys
    for _p in ("/opt/trn_rl_repo", "/root/.axon_site/_ro/trn_rl_repo"):
        if _p not in _sys.path:
            _sys.path.insert(0, _p)
    import concourse.bass as bass
import concourse.mybir as mybir
import concourse.tile as tile
from concourse import bacc

F32 = mybir.dt.float32
F32R = mybir.dt.float32r
BF16 = mybir.dt.bfloat16
FP8 = mybir.dt.float8e4
AX = mybir.AxisListType
ALU = mybir.AluOpType
AF = mybir.ActivationFunctionType
DR = mybir.MatmulPerfMode.DoubleRow

NCORES = 8
B, T, S, D, E, A = 256, 10, 49, 512, 512, 128
N_N = (B // NCORES) * T          # 320 rows per core
ROWS = N_N * S                   # 15680 visual rows per core
VB = 490                         # vblock columns (10 n's)
HALF = 2940                      # visT half size (6 vblocks)
# (n0, n_count, tile_rows) per chunk; rows = n_count*49 divisible by both
# tile_rows and 490.
CHUNKS = [(0, 120, 120), (120, 120, 120), (240, 80, 112)]

# NOTE: GPSIMD (Pool) cannot access PSUM, so psum-reading ops rotate
# over Act/DVE only; Pool gets sbuf->sbuf work (Z-reduce, Att zeroing).
ACT, DVE, POOL = 0, 1, 2
COPY_PAT = [ACT, DVE]
RELU_PAT = [DVE, ACT]
ATT_PAT = [DVE]


def _tiles():
    """Global tile table: (chunk, t, row0_global, tile_rows)."""
    out = []
    for c, (n0c, ncn, tr) in enumerate(CHUNKS):
        rowsc = ncn * S
        for t in range(rowsc // tr):
            out.append((c, t, n0c * S + t * tr, tr))
    return out


TILES = _tiles()
NTILES = len(TILES)              # 133


def build_module():
    nc = bacc.Bacc("TRN2", debug=False)

    aud_d = nc.dram_tensor("audio", [N_N, A], F32R, kind="ExternalInput").ap()
    vis_d = nc.dram_tensor("visual", [ROWS, D], F32R, kind="ExternalInput").ap()
    wat_d = nc.dram_tensor("WaT", [128, E], F32R, kind="ExternalInput").ap()
    aat_d = nc.dram_tensor("AaT256", [128, 4, 64], F32R, kind="ExternalInput").ap()
    wv8_d = nc.dram_tensor("Wv8", [128, 2, 2, 4, 128], F32, kind="ExternalInput").ap()
    av8_d = nc.dram_tensor("Av8", [128, 2, 2, 64], F32, kind="ExternalInput").ap()
    aft_d = nc.dram_tensor("AfT", [S, 1], F32, kind="ExternalInput").ap()
    ones_d = nc.dram_tensor("ones", [1, 64], F32, kind="ExternalInput").ap()
    ba_d = nc.dram_tensor("ba_l", [128, 4], F32, kind="ExternalInput").ap()
    bv_d = nc.dram_tensor("bv32_l", [128, 4], F32, kind="ExternalInput").ap()
    idn_d = nc.dram_tensor("ident", [128, 128], F32R, kind="ExternalInput").ap()
    msk_d = nc.dram_tensor("masks", [128, NTILES, 4], F32, kind="ExternalInput").ap()
    zat_d = nc.dram_tensor("zeros128", [128, 128], F32R, kind="ExternalInput").ap()
    idf_d = nc.dram_tensor("identf1", [1, 1], F32, kind="ExternalInput").ap()
    out_d = nc.dram_tensor("out", [N_N, D], F32, kind="ExternalOutput").ap()

    with tile.TileContext(nc) as tc, \
         tc.tile_pool(name="consts", bufs=1) as cp, \
         tc.tile_pool(name="slab", bufs=3) as slp, \
         tc.tile_pool(name="visT", bufs=2) as vtp, \
         tc.tile_pool(name="vt8", bufs=2) as v8p, \
         tc.tile_pool(name="th", bufs=2) as thp, \
         tc.tile_pool(name="att", bufs=8) as atp, \
         tc.tile_pool(name="erow", bufs=1) as erp, \
         tc.tile_pool(name="asr", bufs=2) as asp, \
         tc.tile_pool(name="outsb", bufs=2) as obp, \
         tc.tile_pool(name="small", bufs=2) as smp, \
         tc.tile_pool(name="dram", bufs=1, space="DRAM") as dp, \
         tc.tile_pool(name="ps_tr", bufs=2, space="PSUM") as ptr, \
         tc.tile_pool(name="ps_mm", bufs=2, space="PSUM") as pmm, \
         tc.tile_pool(name="ps_vs", bufs=2, space="PSUM") as pvs, \
         tc.tile_pool(name="ps_f", bufs=1, space="PSUM") as pft, \
         tc.tile_pool(name="ps_out", bufs=1, space="PSUM") as pou:

        # ---------------- constants ----------------
        wat = cp.tile([128, E], F32R, tag="wat")
        nc.sync.dma_start(wat[:], wat_d)
        aat = cp.tile([128, 4, 64], F32R, tag="aat")
        nc.sync.dma_start(aat[:], aat_d)
        wv8f = cp.tile([128, 2, 2, 4, 128], F32, tag="wv8f")
        nc.sync.dma_start(wv8f[:], wv8_d)
        av8f = cp.tile([128, 2, 2, 64], F32, tag="av8f")
        nc.sync.dma_start(av8f[:], av8_d)
        aftf = cp.tile([S, 1], F32, tag="aftf")
        nc.sync.dma_start(aftf[:], aft_d)
        onesf = cp.tile([1, 64], F32, tag="onesf")
        nc.sync.dma_start(onesf[:], ones_d)
        ba = cp.tile([128, 4], F32, tag="ba")
        nc.sync.dma_start(ba[:], ba_d)
        bv32 = cp.tile([128, 4], F32, tag="bv32")
        nc.sync.dma_start(bv32[:], bv_d)
        idn = cp.tile([128, 128], F32R, tag="idn")
        nc.sync.dma_start(idn[:], idn_d)
        masks = cp.tile([128, NTILES, 4], F32, tag="masks")
        nc.sync.dma_start(masks[:], msk_d)
        zat = cp.tile([128, 128], F32R, tag="zat")
        nc.sync.dma_start(zat[:], zat_d)
        idf = cp.tile([1, 1], F32, tag="idf")
        nc.sync.dma_start(idf[:], idf_d)

        # on-chip casts of the small weights
        wv8 = cp.tile([128, 2, 2, 4, 128], FP8, tag="wv8")
        nc.scalar.activation(wv8[:], wv8f[:], AF.Copy)
        av8 = cp.tile([128, 2, 2, 64], FP8, tag="av8")
        nc.scalar.activation(av8[:], av8f[:], AF.Copy)
        aftb = cp.tile([S, 1], BF16, tag="aftb")
        nc.scalar.activation(aftb[:], aftf[:], AF.Copy)
        onesb = cp.tile([1, 64], BF16, tag="onesb")
        nc.scalar.activation(onesb[:], onesf[:], AF.Copy)

        att_bufs = []
        for i in range(8):
            ab = atp.tile([128, 128], F32R, tag="att", name=f"att{i}")
            nc.sync.dma_start(ab[:], zat_d)
            att_bufs.append(ab)
        att_cols = [None] * 8           # (jc0, m) of last use per buf

        audT = cp.tile([128, N_N], F32R, tag="audT")
        atT = cp.tile([128, 4, N_N], F32R, tag="atT")
        zrow = cp.tile([1, 128], F32, tag="zrow")
        as_dram = dp.tile([1, ROWS], BF16, tag="asd")

        # engine-rotating elementwise helpers ------------------------------
        def cast_copy(eng, dst, src, scale=None):
            if eng == ACT:
                if scale is None:
                    nc.scalar.activation(dst, src, AF.Copy)
                else:
                    nc.scalar.activation(dst, src, AF.Copy, scale=scale)
            else:
                e = nc.vector if eng == DVE else nc.gpsimd
                if scale is None:
                    e.tensor_copy(dst, src)
                else:
                    e.tensor_scalar(dst, src, float(scale), None, ALU.mult)

        def relu_op(eng, dst, src, bias_ap):
            if eng == ACT:
                nc.scalar.activation(dst, src, AF.Relu, bias=bias_ap)
            else:
                e = nc.vector if eng == DVE else nc.gpsimd
                e.tensor_scalar(dst, src, bias_ap, 0.0, ALU.add, ALU.max)

        copy_i = [0]
        relu_i = [0]
        att_i = [0]

        def rot(pat, i):
            e = pat[i[0] % len(pat)]
            i[0] += 1
            return e

        # ---------------- audio prologue ----------------
        for it in range(3):
            n0 = it * 128
            nr = min(128, N_N - n0)
            an = smp.tile([128, A], F32R, tag="an")
            nc.sync.dma_start(an[:nr, :], aud_d[n0:n0 + nr, :])
            ps = ptr.tile([128, 4, 128], F32, tag="tr")
            nc.tensor.transpose(ps[:, 0, :nr].bitcast(F32R), an[:nr, :],
                                idn[:nr, :nr])
            nc.scalar.copy(audT[:, n0:n0 + nr], ps[:, 0, :nr])

        for eo in range(4):
            ps = pmm.tile([128, VB], F32, tag="mm")
            nc.tensor.matmul(ps[:, :N_N], wat[:, eo * 128:(eo + 1) * 128],
                             audT[:], start=True, stop=True)
            nc.scalar.activation(atT[:, eo, :], ps[:, :N_N], AF.Relu,
                                 bias=ba[:, eo:eo + 1])

        for it in range(3):
            n0 = it * 128
            nr = min(128, N_N - n0)
            ps = pmm.tile([128, VB], F32, tag="mm")
            for eo in range(4):
                nc.tensor.matmul(ps[:nr, :64], atT[:, eo, n0:n0 + nr],
                                 aat[:, eo, :], start=(eo == 0), stop=(eo == 3))
            asn = smp.tile([128, S], BF16, tag="asn")
            nc.scalar.copy(asn[:nr, :], ps[:nr, :S])
            dst = as_dram[0:1, n0 * S:(n0 + nr) * S]
            nc.sync.dma_start(dst.rearrange("one (n s) -> (one n) s", s=S),
                              asn[:nr, :])

        # ---------------- main loop ----------------
        gt0 = 0  # global tile index of current chunk's tile 0
        for c, (n0c, ncn, trh) in enumerate(CHUNKS):
            rowsc = ncn * S
            row0c = n0c * S
            ntile = rowsc // trh
            nvb = rowsc // VB

            asr = asp.tile([1, 6 * VB * 2], BF16, tag="asr")
            nc.sync.dma_start(asr[0:1, 0:rowsc], as_dram[0:1, row0c:row0c + rowsc])
            erow = erp.tile([1, 6 * VB * 2], F32, tag="erow")
            pso = pou.tile([128, D], F32, tag="out")
            tps = pft.tile([128, 497], F32, tag="fps")
            halves = [vtp.tile([128, 4, HALF], FP8, tag="visT", name=f"visT{c}_{h}")
                      for h in range(2)]
            slabs = {}

            def emit_vblock(vb):
                h = vb // 6
                cs = vb * VB - h * HALF
                vt8 = v8p.tile([128, 2, 2, VB], FP8, tag="vt8")
                for eo in range(4):
                    psm = pmm.tile([128, VB], F32, tag="mm")
                    for h2 in range(2):
                        nc.tensor.matmul(psm[:], wv8[:, h2, :, eo, :],
                                         halves[h][:, 2 * h2:2 * h2 + 2, cs:cs + VB],
                                         start=(h2 == 0), stop=(h2 == 1),
                                         perf_mode=DR)
                    relu_op(rot(RELU_PAT, relu_i), vt8[:, eo // 2, eo % 2, :],
                            psm[:], bv32[:, eo:eo + 1])
                psv = pvs.tile([64, VB], F32, tag="vs")
                for h2 in range(2):
                    nc.tensor.matmul(psv[:], av8[:, h2, :, :], vt8[:, h2, :, :],
                                     start=(h2 == 0), stop=False, perf_mode=DR)
                nc.tensor.matmul(psv[:], onesb[:],
                                 asr[0:1, vb * VB:(vb + 1) * VB],
                                 start=False, stop=True)
                th = thp.tile([S, VB], BF16, tag="th")
                nc.scalar.activation(th[:], psv[:S, :], AF.Tanh,
                                     scale=1.0 / 256.0)
                nc.tensor.matmul(tps[0:1, 0:VB], aftb[:], th[:],
                                 start=True, stop=True)
                nc.scalar.activation(erow[0:1, vb * VB:(vb + 1) * VB],
                                     tps[0:1, 0:VB], AF.Exp)
                nc.vector.reduce_sum(
                    zrow[0:1, vb * 10:(vb + 1) * 10],
                    erow[0:1, vb * VB:(vb + 1) * VB]
                    .rearrange("p (n s) -> p n s", s=S), axis=AX.X)

            def emit_eta(t):
                """e-column transpose + Att build for tile t."""
                gt = gt0 + t
                tr = trh
                j = t % 6
                ecol = tps[:, 490 + j:491 + j]
                nc.tensor.transpose(ecol[0:tr, :],
                                    erow[0:1, t * tr:t * tr + tr],
                                    idf[0:1, 0:1])
                row0 = TILES[gt][2]
                jc0 = row0 // S - n0c
                m = (row0 + tr - 1) // S - row0 // S + 1
                ab = att_bufs[gt % 8]
                stale = att_cols[gt % 8]
                if stale is not None:
                    s0, s1 = stale
                    nc.gpsimd.tensor_copy(ab[:, s0:s1], zat[:, 0:s1 - s0])
                att_cols[gt % 8] = (jc0, jc0 + m)
                nc.vector.tensor_tensor(ab[0:tr, jc0:jc0 + m],
                                        masks[0:tr, gt, 0:m],
                                        ecol[0:tr, 0:1].broadcast_to([tr, m]),
                                        ALU.mult)

            def emit_out(t):
                slab = slabs[t // 7]
                tr = trh
                nc.tensor.matmul(pso[:], att_bufs[(gt0 + t) % 8][0:tr, :],
                                 slab[0:tr, t % 7, :],
                                 start=(t == 0), stop=(t == ntile - 1))

            state = {"vb": 0, "eta": 0, "out": 0}

            def emit_eta_guarded():
                # att buf k%8 is recycled at eta(k+8): out(k) must be
                # emitted before eta(k+8) clobbers its values.
                while state["out"] <= state["eta"] - 8:
                    emit_out(state["out"])
                    state["out"] += 1
                emit_eta(state["eta"])
                state["eta"] += 1

            def emit_tile(t):
                if t % 7 == 0:
                    sl = slp.tile([trh, 7, D], F32R, tag="slab",
                                  name=f"slab{c}_{t // 7}")
                    slabs[t // 7] = sl
                    r0 = row0c + t * trh
                    src = vis_d[r0:r0 + 7 * trh, :]
                    nc.sync.dma_start(
                        sl[:], src.rearrange("(j p) d -> p j d", j=7))
                ps = ptr.tile([128, 4, 128], F32, tag="tr")
                for do in range(4):
                    nc.tensor.transpose(ps[:, do, :trh].bitcast(F32R),
                                        slabs[t // 7][0:trh, t % 7,
                                                      do * 128:(do + 1) * 128],
                                        idn[:trh, :trh])
                c0 = t * trh
                c1 = c0 + trh
                for h in range(2):
                    lo = max(c0, h * HALF)
                    hi = min(c1, HALF + h * (rowsc - HALF))
                    if lo < hi:
                        cast_copy(rot(COPY_PAT, copy_i),
                                  halves[h][:, :, lo - h * HALF:hi - h * HALF],
                                  ps[:, :, lo - c0:hi - c0], scale=4.0)

            for t in range(ntile):
                emit_tile(t)
                while (state["vb"] + 1) * VB <= (t + 1) * trh:
                    emit_vblock(state["vb"])
                    state["vb"] += 1
                    while (state["eta"] + 1) * trh <= (state["vb"] - 1) * VB:
                        emit_eta_guarded()
                    while state["out"] < state["eta"] and \
                            (state["out"] + 1) * trh <= (state["vb"] - 2) * VB:
                        emit_out(state["out"])
                        state["out"] += 1
            while state["eta"] < ntile:
                emit_eta_guarded()
            while state["out"] < ntile:
                emit_out(state["out"])
                state["out"] += 1

            # chunk epilogue: rinv + scaled copy + store
            nc.tensor.transpose(tps[0:ncn, 496:497],
                                zrow[0:1, 0:ncn], idf[0:1, 0:1])
            zcol = smp.tile([128, 1], F32, tag="zcol")
            nc.vector.tensor_copy(zcol[0:ncn, :], tps[0:ncn, 496:497])
            rin = smp.tile([128, 1], F32, tag="rin")
            nc.vector.reciprocal(rin[0:ncn, :], zcol[0:ncn, :])
            ob = obp.tile([128, D], F32, tag="ob")
            nc.scalar.activation(ob[0:ncn, :], pso[0:ncn, :], AF.Copy,
                                 scale=rin[0:ncn, 0:1])
            nc.sync.dma_start(out_d[n0c:n0c + ncn, :], ob[0:ncn, :])
            gt0 += ntile

    nc.finalize()
    return nc


def prep_consts(Wa, ba_, Wv, bv_, Aa, Av, Af):
    f = np.float32
    c = {}
    c["WaT"] = np.ascontiguousarray(Wa.T).astype(f)
    aat = np.zeros((128, 4, 64), f)
    # aat[p, eo, s] = 256*Aa[s, eo*128+p]
    aat[:, :, :S] = (256.0 * Aa.T).reshape(4, 128, S).transpose(1, 0, 2)
    c["AaT256"] = aat
    # wv8[p, h, i, eo, m] = 8*Wv[eo*128+m, h*256+i*128+p]
    w = 8.0 * Wv.astype(f)                   # [e, d]
    w = w.T.reshape(2, 2, 128, 4, 128)       # [h, i, p, eo, m]
    c["Wv8"] = np.ascontiguousarray(w.transpose(2, 0, 1, 3, 4))
    a = np.zeros((64, 512), f)               # pad f-dim to 64 (DR needs M=64/128)
    a[:S] = 8.0 * Av.astype(f)
    a = a.T.reshape(2, 2, 128, 64)           # [h, i, p, f]
    c["Av8"] = np.ascontiguousarray(a.transpose(2, 0, 1, 3))
    c["AfT"] = np.ascontiguousarray(Af.reshape(1, S).T).astype(f)
    c["ones"] = np.ones((1, 64), f)
    c["ba_l"] = np.ascontiguousarray(ba_.reshape(4, 128).T).astype(f)
    c["bv32_l"] = np.ascontiguousarray(32.0 * bv_.reshape(4, 128).T).astype(f)
    c["ident"] = np.eye(128, dtype=f)
    c["identf1"] = np.ones((1, 1), f)
    c["zeros128"] = np.zeros((128, 128), f)
    masks = np.zeros((128, NTILES, 4), f)
    for gt, (cc, t, row0, tr) in enumerate(TILES):
        n0 = row0 // S
        for p in range(tr):
            j = (row0 + p) // S - n0
            masks[p, gt, j] = 1.0
    c["masks"] = masks
    return c


_CACHE = {}


def kernel(audio, visual, Wa, ba, Wv, bv, Aa, Av, Af):
    from concourse.bass_utils import run_bass_kernel_spmd

    audio = np.asarray(audio, np.float32)
    visual = np.asarray(visual, np.float32)

    if "nc" not in _CACHE:
        _CACHE["nc"] = build_module()
    nc = _CACHE["nc"]

    consts = prep_consts(np.asarray(Wa, np.float32), np.asarray(ba, np.float32),
                         np.asarray(Wv, np.float32), np.asarray(bv, np.float32),
                         np.asarray(Aa, np.float32), np.asarray(Av, np.float32),
                         np.asarray(Af, np.float32))
    bs = B // NCORES
    in_maps = []
    for cid in range(NCORES):
        m = dict(consts)
        m["audio"] = np.ascontiguousarray(
            audio[cid * bs:(cid + 1) * bs].reshape(N_N, A))
        m["visual"] = np.ascontiguousarray(
            visual[cid * bs:(cid + 1) * bs].reshape(ROWS, D))
        in_maps.append(m)

    res = run_bass_kernel_spmd(nc, in_maps, core_ids=list(range(NCORES)))
    _CACHE["last_res"] = res
    out = np.concatenate(
        [r["out"].reshape(bs, T, D) for r in res.results], axis=0)
    return out.astype(np.float32)


# revision 24
# speedup vs baseline: 1.2349x; 1.1822x over previous
"""Trainium2 Bass kernel for nn_AttentionNet (audio-visual attention).

Data-parallel across 8 NeuronCores: B=256 split 32/core -> 320 (b,t) rows
("n") and 15680 visual rows per core.

Per-core math (n in [0,320), s in [0,49), d/e in [0,512)):
    a_t = relu(audio @ Wa.T + ba)            [N,512]
    a_s = a_t @ Aa.T                         [N,49]
    v_t = relu(vis @ Wv.T + bv)              [N,49,512]
    v_s = v_t @ Av.T                         [N,49,49]
    f   = tanh(a_s[:,:,None] + v_s) @ Af.T   [N,49]
    att = softmax_s(f);  out = att @ vis     [N,512]

Implementation notes:
  * v_t / v_s run as fp8e4 DoubleRow matmuls (0.5 cyc/row, K=256/instr).
    Scales: visT holds 4*vis, wv8 holds 8*Wv -> psum = 32*pre-act;
    vt8 = relu(psum + 32*bv) = 32*v_t; av8 = 8*Av -> v_s psum = 256*v_s;
    a_s enters the same psum scaled by 256 (host-scaled Aa); tanh uses
    scale=1/256.
  * Work is chunked by 120/120/80 "n" rows (psum partition limit) with
    row-tiles of 120/120/112 and 490-column vblocks (49*10, n-aligned).
  * out = att @ vis runs on the PE against the *untransposed* vis tiles:
    per row-tile a masked attention matrix Att[row, n'] (e values scattered
    to each row's n-column) is built from a PE column-transpose of the exp
    row + a small mask multiply; one accumulating matmul per tile.
  * Softmax is unnormalized; 1/Z folds into the final psum->sbuf copy.
  * Elementwise work (psum copies / relu) rotates across Act, DVE and
    GpSimd(Pool) engines to keep all three below the PE/DMA roofline.
"""

import numpy as np

try:
    import concourse.bass as bass
except ImportError:
    import sys as _sys
    for _p in ("/opt/trn_rl_repo", "/root/.axon_site/_ro/trn_rl_repo"):
        if _p not in _sys.path:
            _sys.path.insert(0, _p)
    import concourse.bass as bass
import concourse.mybir as mybir
import concourse.tile as tile
from concourse import bacc

F32 = mybir.dt.float32
F32R = mybir.dt.float32r
BF16 = mybir.dt.bfloat16
FP8 = mybir.dt.float8e4
AX = mybir.AxisListType
ALU = mybir.AluOpType
AF = mybir.ActivationFunctionType
DR = mybir.MatmulPerfMode.DoubleRow

NCORES = 8
B, T, S, D, E, A = 256, 10, 49, 512, 512, 128
N_N = (B // NCORES) * T          # 320 rows per core
ROWS = N_N * S                   # 15680 visual rows per core
VB = 490                         # vblock columns (10 n's)
HALF = 2940                      # visT half size (6 vblocks)
# (n0, n_count, tile_rows) per chunk; rows = n_count*49 divisible by both
# tile_rows and 490.
CHUNKS = [(0, 120, 120), (120, 120, 120), (240, 80, 112)]

# NOTE: GPSIMD (Pool) cannot access PSUM, so psum-reading ops rotate
# over Act/DVE only; Pool gets sbuf->sbuf work (Z-reduce, Att zeroing).
ACT, DVE, POOL = 0, 1, 2
COPY_PAT = [ACT, DVE]
RELU_PAT = [DVE, ACT]
ATT_PAT = [DVE]


def _tiles():
    """Global tile table: (chunk, t, row0_global, tile_rows)."""
    out = []
    for c, (n0c, ncn, tr) in enumerate(CHUNKS):
        rowsc = ncn * S
        for t in range(rowsc // tr):
            out.append((c, t, n0c * S + t * tr, tr))
    return out


TILES = _tiles()
NTILES = len(TILES)              # 133


def build_module():
    nc = bacc.Bacc("TRN2", debug=False)

    aud_d = nc.dram_tensor("audio", [N_N, A], F32R, kind="ExternalInput").ap()
    vis_d = nc.dram_tensor("visual", [ROWS, D], F32R, kind="ExternalInput").ap()
    wat_d = nc.dram_tensor("WaT", [128, E], F32R, kind="ExternalInput").ap()
    aat_d = nc.dram_tensor("AaT256", [128, 4, 64], F32R, kind="ExternalInput").ap()
    wv8_d = nc.dram_tensor("Wv8", [128, 2, 2, 4, 128], F32, kind="ExternalInput").ap()
    av8_d = nc.dram_tensor("Av8", [128, 2, 2, 64], F32, kind="ExternalInput").ap()
    aft_d = nc.dram_tensor("AfT", [S, 1], F32, kind="ExternalInput").ap()
    ones_d = nc.dram_tensor("ones", [1, 64], F32, kind="ExternalInput").ap()
    ba_d = nc.dram_tensor("ba_l", [128, 4], F32, kind="ExternalInput").ap()
    bv_d = nc.dram_tensor("bv32_l", [128, 4], F32, kind="ExternalInput").ap()
    idn_d = nc.dram_tensor("ident", [128, 128], F32R, kind="ExternalInput").ap()
    msk_d = nc.dram_tensor("masks", [128, NTILES, 4], F32, kind="ExternalInput").ap()
    zat_d = nc.dram_tensor("zeros128", [128, 128], F32R, kind="ExternalInput").ap()
    idf_d = nc.dram_tensor("identf1", [1, 1], F32, kind="ExternalInput").ap()
    out_d = nc.dram_tensor("out", [N_N, D], F32, kind="ExternalOutput").ap()

    with tile.TileContext(nc) as tc, \
         tc.tile_pool(name="consts", bufs=1) as cp, \
         tc.tile_pool(name="slab", bufs=3) as slp, \
         tc.tile_pool(name="visT", bufs=2) as vtp, \
         tc.tile_pool(name="vt8", bufs=2) as v8p, \
         tc.tile_pool(name="th", bufs=2) as thp, \
         tc.tile_pool(name="att", bufs=8) as atp, \
         tc.tile_pool(name="erow", bufs=2) as erp, \
         tc.tile_pool(name="asr", bufs=2) as asp, \
         tc.tile_pool(name="ecs", bufs=6) as ecp, \
         tc.tile_pool(name="outsb", bufs=2) as obp, \
         tc.tile_pool(name="small", bufs=2) as smp, \
         tc.tile_pool(name="dram", bufs=1, space="DRAM") as dp, \
         tc.tile_pool(name="ps_tr", bufs=2, space="PSUM") as ptr, \
         tc.tile_pool(name="ps_mm", bufs=2, space="PSUM") as pmm, \
         tc.tile_pool(name="ps_vs", bufs=2, space="PSUM") as pvs, \
         tc.tile_pool(name="ps_f", bufs=1, space="PSUM") as pft, \
         tc.tile_pool(name="ps_out", bufs=1, space="PSUM") as pou:

        # ------- small constants needed by the audio prologue (early) -------
        idn = cp.tile([128, 128], F32R, tag="idn")
        nc.sync.dma_start(idn[:], idn_d)
        idf = cp.tile([1, 1], F32, tag="idf")
        nc.sync.dma_start(idf[:], idf_d)
        wat = cp.tile([128, E], F32R, tag="wat")
        nc.sync.dma_start(wat[:], wat_d)
        aat = cp.tile([128, 4, 64], F32R, tag="aat")
        nc.sync.dma_start(aat[:], aat_d)
        ba = cp.tile([128, 4], F32, tag="ba")
        nc.sync.dma_start(ba[:], ba_d)
        bv32 = cp.tile([128, 4], F32, tag="bv32")
        nc.sync.dma_start(bv32[:], bv_d)
        aftf = cp.tile([S, 1], F32, tag="aftf")
        nc.sync.dma_start(aftf[:], aft_d)
        onesf = cp.tile([1, 64], F32, tag="onesf")
        nc.sync.dma_start(onesf[:], ones_d)

        audT = cp.tile([128, N_N], F32R, tag="audT")
        atT = cp.tile([128, 4, N_N], F32R, tag="atT")
        zrow = cp.tile([1, 128], F32, tag="zrow")
        as_dram = dp.tile([1, ROWS], BF16, tag="asd")

        # engine-rotating elementwise helpers ------------------------------
        def cast_copy(eng, dst, src, scale=None):
            if eng == ACT:
                if scale is None:
                    nc.scalar.activation(dst, src, AF.Copy)
                else:
                    nc.scalar.activation(dst, src, AF.Copy, scale=scale)
            else:
                e = nc.vector if eng == DVE else nc.gpsimd
                if scale is None:
                    e.tensor_copy(dst, src)
                else:
                    e.tensor_scalar(dst, src, float(scale), None, ALU.mult)

        def relu_op(eng, dst, src, bias_ap):
            if eng == ACT:
                nc.scalar.activation(dst, src, AF.Relu, bias=bias_ap)
            else:
                e = nc.vector if eng == DVE else nc.gpsimd
                e.tensor_scalar(dst, src, bias_ap, 0.0, ALU.add, ALU.max)

        copy_i = [0]
        relu_i = [0]
        att_i = [0]

        def rot(pat, i):
            e = pat[i[0] % len(pat)]
            i[0] += 1
            return e

        # ---------------- audio prologue ----------------
        for it in range(3):
            n0 = it * 128
            nr = min(128, N_N - n0)
            an = smp.tile([128, A], F32R, tag="an")
            nc.sync.dma_start(an[:nr, :], aud_d[n0:n0 + nr, :])
            ps = ptr.tile([128, 4, 128], F32, tag="tr")
            nc.tensor.transpose(ps[:, 0, :nr].bitcast(F32R), an[:nr, :],
                                idn[:nr, :nr])
            nc.scalar.copy(audT[:, n0:n0 + nr], ps[:, 0, :nr])

        for eo in range(4):
            ps = pmm.tile([128, VB], F32, tag="mm")
            nc.tensor.matmul(ps[:, :N_N], wat[:, eo * 128:(eo + 1) * 128],
                             audT[:], start=True, stop=True)
            nc.scalar.activation(atT[:, eo, :], ps[:, :N_N], AF.Relu,
                                 bias=ba[:, eo:eo + 1])

        for it in range(3):
            n0 = it * 128
            nr = min(128, N_N - n0)
            ps = pmm.tile([128, VB], F32, tag="mm")
            for eo in range(4):
                nc.tensor.matmul(ps[:nr, :64], atT[:, eo, n0:n0 + nr],
                                 aat[:, eo, :], start=(eo == 0), stop=(eo == 3))
            asn = smp.tile([128, S], BF16, tag="asn")
            nc.scalar.copy(asn[:nr, :], ps[:nr, :S])
            dst = as_dram[0:1, n0 * S:(n0 + nr) * S]
            nc.gpsimd.dma_start(dst.rearrange("one (n s) -> (one n) s", s=S),
                                asn[:nr, :])

        # ------- fat constants (not needed in the first ~8us) -------
        wv8f = cp.tile([128, 2, 2, 4, 128], F32, tag="wv8f")
        nc.sync.dma_start(wv8f[:], wv8_d)
        av8f = cp.tile([128, 2, 2, 64], F32, tag="av8f")
        nc.sync.dma_start(av8f[:], av8_d)
        masks = cp.tile([128, NTILES, 4], F32, tag="masks")
        nc.sync.dma_start(masks[:], msk_d)
        zat = cp.tile([128, 128], F32R, tag="zat")
        nc.sync.dma_start(zat[:], zat_d)

        wv8 = cp.tile([128, 2, 2, 4, 128], FP8, tag="wv8")
        nc.scalar.activation(wv8[:], wv8f[:], AF.Copy)
        av8 = cp.tile([128, 2, 2, 64], FP8, tag="av8")
        nc.scalar.activation(av8[:], av8f[:], AF.Copy)
        aftb = cp.tile([S, 1], BF16, tag="aftb")
        nc.scalar.activation(aftb[:], aftf[:], AF.Copy)
        onesb = cp.tile([1, 64], BF16, tag="onesb")
        nc.scalar.activation(onesb[:], onesf[:], AF.Copy)

        att_bufs = []
        for i in range(8):
            ab = atp.tile([128, 128], F32R, tag="att", name=f"att{i}")
            nc.sync.dma_start(ab[:], zat_d)
            att_bufs.append(ab)
        att_cols = [None] * 8           # (jc0, m) of last use per buf

        # shared f/e-col/rinv psum bank: f row [0:1, 0:490], e-col slots
        # 490..497 (4 per vblock, vb%2 parity), rinv transpose col 498.
        tps = pft.tile([128, 499], F32, tag="fps")
        nc.vector.memset(tps[:], 0.0)

        # ---------------- main loop ----------------
        gt0 = 0  # global tile index of current chunk's tile 0
        for c, (n0c, ncn, trh) in enumerate(CHUNKS):
            rowsc = ncn * S
            row0c = n0c * S
            ntile = rowsc // trh
            nvb = rowsc // VB

            asr = asp.tile([1, 6 * VB * 2], BF16, tag="asr")
            nc.gpsimd.dma_start(asr[0:1, 0:rowsc],
                                as_dram[0:1, row0c:row0c + rowsc])
            pso = pou.tile([128, D], F32, tag="out")
            erow = erp.tile([1, 6 * VB * 2], F32, tag="erow")
            halves = [vtp.tile([128, 4, HALF], FP8, tag="visT", name=f"visT{c}_{h}")
                      for h in range(2)]
            slabs = {}

            def emit_vblock(vb):
                h = vb // 6
                cs = vb * VB - h * HALF
                vt8 = v8p.tile([128, 2, 2, VB], FP8, tag="vt8")
                for eo in range(4):
                    psm = pmm.tile([128, VB], F32, tag="mm")
                    for h2 in range(2):
                        nc.tensor.matmul(psm[:], wv8[:, h2, :, eo, :],
                                         halves[h][:, 2 * h2:2 * h2 + 2, cs:cs + VB],
                                         start=(h2 == 0), stop=(h2 == 1),
                                         perf_mode=DR)
                    relu_op(rot(RELU_PAT, relu_i), vt8[:, eo // 2, eo % 2, :],
                            psm[:], bv32[:, eo:eo + 1])
                psv = pvs.tile([64, VB], F32, tag="vs")
                for h2 in range(2):
                    nc.tensor.matmul(psv[:], av8[:, h2, :, :], vt8[:, h2, :, :],
                                     start=(h2 == 0), stop=False, perf_mode=DR)
                nc.tensor.matmul(psv[:], onesb[:],
                                 asr[0:1, vb * VB:(vb + 1) * VB],
                                 start=False, stop=True)
                th = thp.tile([S, VB], BF16, tag="th")
                nc.scalar.activation(th[:], psv[:S, :], AF.Tanh,
                                     scale=1.0 / 256.0)
                nc.tensor.matmul(tps[0:1, 0:VB], aftb[:], th[:],
                                 start=True, stop=True)
                nc.scalar.activation(erow[0:1, vb * VB:(vb + 1) * VB],
                                     tps[0:1, 0:VB], AF.Exp)
                nc.vector.reduce_sum(
                    zrow[0:1, vb * 10:(vb + 1) * 10],
                    erow[0:1, vb * VB:(vb + 1) * VB]
                    .rearrange("p (n s) -> p n s", s=S), axis=AX.X)

            def emit_eta(t):
                """e-col transpose (PE) -> sbuf stage (DVE) -> Att (Pool)."""
                gt = gt0 + t
                tr = trh
                j = t % 6
                ecol = tps[:, 490 + j:491 + j]
                nc.tensor.transpose(ecol[0:tr, :],
                                    erow[0:1, t * tr:t * tr + tr],
                                    idf[0:1, 0:1])
                ecs = ecp.tile([128, 1], F32, tag="ecs")
                nc.vector.tensor_copy(ecs[0:tr, :], ecol[0:tr, :])
                row0 = TILES[gt][2]
                jc0 = row0 // S - n0c
                m = (row0 + tr - 1) // S - row0 // S + 1
                ab = att_bufs[gt % 8]
                stale = att_cols[gt % 8]
                if stale is not None:
                    s0, s1 = stale
                    nc.gpsimd.tensor_copy(ab[:, s0:s1], zat[:, 0:s1 - s0])
                att_cols[gt % 8] = (jc0, jc0 + m)
                nc.gpsimd.tensor_tensor(ab[0:tr, jc0:jc0 + m],
                                        masks[0:tr, gt, 0:m],
                                        ecs[0:tr, 0:1].broadcast_to([tr, m]),
                                        ALU.mult)

            def emit_out(t):
                slab = slabs[t // 7]
                tr = trh
                nc.tensor.matmul(pso[:], att_bufs[(gt0 + t) % 8][0:tr, :],
                                 slab[0:tr, t % 7, :],
                                 start=(t == 0), stop=(t == ntile - 1))

            state = {"vb": 0, "eta": 0, "out": 0}

            def emit_vblock_seq():
                vb = state["vb"]
                # flush etas/outs BEFORE this vblock's ops: their erow
                # reads then depend on exp(vb-1), already one vblock old.
                while (state["eta"] + 1) * trh <= vb * VB:
                    emit_eta_guarded()
                while state["out"] < state["eta"] and \
                        (state["out"] + 1) * trh <= (vb - 1) * VB:
                    emit_out(state["out"])
                    state["out"] += 1
                emit_vblock(vb)
                state["vb"] += 1

            def emit_eta_guarded():
                # att buf k%8 is recycled at eta(k+8): out(k) must be
                # emitted before eta(k+8) clobbers its values.
                while state["out"] <= state["eta"] - 8:
                    emit_out(state["out"])
                    state["out"] += 1
                emit_eta(state["eta"])
                state["eta"] += 1

            def emit_tile(t):
                if t % 7 == 0:
                    sl = slp.tile([trh, 7, D], F32R, tag="slab",
                                  name=f"slab{c}_{t // 7}")
                    slabs[t // 7] = sl
                    r0 = row0c + t * trh
                    src = vis_d[r0:r0 + 7 * trh, :]
                    nc.sync.dma_start(
                        sl[:], src.rearrange("(j p) d -> p j d", j=7))
                ps = ptr.tile([128, 4, 128], F32, tag="tr")
                for do in range(4):
                    nc.tensor.transpose(ps[:, do, :trh].bitcast(F32R),
                                        slabs[t // 7][0:trh, t % 7,
                                                      do * 128:(do + 1) * 128],
                                        idn[:trh, :trh])
                c0 = t * trh
                c1 = c0 + trh
                for h in range(2):
                    lo = max(c0, h * HALF)
                    hi = min(c1, HALF + h * (rowsc - HALF))
                    if lo < hi:
                        cast_copy(rot(COPY_PAT, copy_i),
                                  halves[h][:, :, lo - h * HALF:hi - h * HALF],
                                  ps[:, :, lo - c0:hi - c0], scale=4.0)

            def pump(t):
                # one extra tile of lag so visT copies clear the Act/DVE
                # queues before the PE needs them
                while (state["vb"] + 1) * VB <= t * trh:
                    emit_vblock_seq()

            for t in range(ntile):
                emit_tile(t)
                pump(t)
            while state["vb"] < nvb:
                emit_vblock_seq()
            while state["eta"] < ntile:
                emit_eta_guarded()
            while state["out"] < ntile:
                emit_out(state["out"])
                state["out"] += 1

            # chunk epilogue: rinv + scaled copy + store
            nc.tensor.transpose(tps[0:ncn, 498:499],
                                zrow[0:1, 0:ncn], idf[0:1, 0:1])
            zcol = smp.tile([128, 1], F32, tag="zcol")
            nc.vector.tensor_copy(zcol[0:ncn, :], tps[0:ncn, 498:499])
            rin = smp.tile([128, 1], F32, tag="rin")
            nc.vector.reciprocal(rin[0:ncn, :], zcol[0:ncn, :])
            ob = obp.tile([128, D], F32, tag="ob")
            nc.scalar.activation(ob[0:ncn, :], pso[0:ncn, :], AF.Copy,
                                 scale=rin[0:ncn, 0:1])
            nc.gpsimd.dma_start(out_d[n0c:n0c + ncn, :], ob[0:ncn, :])
            gt0 += ntile

    nc.finalize()
    return nc


def prep_consts(Wa, ba_, Wv, bv_, Aa, Av, Af):
    f = np.float32
    c = {}
    c["WaT"] = np.ascontiguousarray(Wa.T).astype(f)
    aat = np.zeros((128, 4, 64), f)
    # aat[p, eo, s] = 256*Aa[s, eo*128+p]
    aat[:, :, :S] = (256.0 * Aa.T).reshape(4, 128, S).transpose(1, 0, 2)
    c["AaT256"] = aat
    # wv8[p, h, i, eo, m] = 8*Wv[eo*128+m, h*256+i*128+p]
    w = 8.0 * Wv.astype(f)                   # [e, d]
    w = w.T.reshape(2, 2, 128, 4, 128)       # [h, i, p, eo, m]
    c["Wv8"] = np.ascontiguousarray(w.transpose(2, 0, 1, 3, 4))
    a = np.zeros((64, 512), f)               # pad f-dim to 64 (DR needs M=64/128)
    a[:S] = 8.0 * Av.astype(f)
    a = a.T.reshape(2, 2, 128, 64)           # [h, i, p, f]
    c["Av8"] = np.ascontiguousarray(a.transpose(2, 0, 1, 3))
    c["AfT"] = np.ascontiguousarray(Af.reshape(1, S).T).astype(f)
    c["ones"] = np.ones((1, 64), f)
    c["ba_l"] = np.ascontiguousarray(ba_.reshape(4, 128).T).astype(f)
    c["bv32_l"] = np.ascontiguousarray(32.0 * bv_.reshape(4, 128).T).astype(f)
    c["ident"] = np.eye(128, dtype=f)
    c["identf1"] = np.ones((1, 1), f)
    c["zeros128"] = np.zeros((128, 128), f)
    masks = np.zeros((128, NTILES, 4), f)
    for gt, (cc, t, row0, tr) in enumerate(TILES):
        n0 = row0 // S
        for p in range(tr):
            j = (row0 + p) // S - n0
            masks[p, gt, j] = 1.0
    c["masks"] = masks
    return c


_CACHE = {}


def kernel(audio, visual, Wa, ba, Wv, bv, Aa, Av, Af):
    from concourse.bass_utils import run_bass_kernel_spmd

    audio = np.asarray(audio, np.float32)
    visual = np.asarray(visual, np.float32)

    if "nc" not in _CACHE:
        _CACHE["nc"] = build_module()
    nc = _CACHE["nc"]

    consts = prep_consts(np.asarray(Wa, np.float32), np.asarray(ba, np.float32),
                         np.asarray(Wv, np.float32), np.asarray(bv, np.float32),
                         np.asarray(Aa, np.float32), np.asarray(Av, np.float32),
                         np.asarray(Af, np.float32))
    bs = B // NCORES
    in_maps = []
    for cid in range(NCORES):
        m = dict(consts)
        m["audio"] = np.ascontiguousarray(
            audio[cid * bs:(cid + 1) * bs].reshape(N_N, A))
        m["visual"] = np.ascontiguousarray(
            visual[cid * bs:(cid + 1) * bs].reshape(ROWS, D))
        in_maps.append(m)

    res = run_bass_kernel_spmd(nc, in_maps, core_ids=list(range(NCORES)))
    _CACHE["last_res"] = res
    out = np.concatenate(
        [r["out"].reshape(bs, T, D) for r in res.results], axis=0)
    return out.astype(np.float32)


# revision 26
# speedup vs baseline: 1.2501x; 1.0123x over previous
"""Trainium2 Bass kernel for nn_AttentionNet (audio-visual attention).

Data-parallel across 8 NeuronCores: B=256 split 32/core -> 320 (b,t) rows
("n") and 15680 visual rows per core.

Per-core math (n in [0,320), s in [0,49), d/e in [0,512)):
    a_t = relu(audio @ Wa.T + ba)            [N,512]
    a_s = a_t @ Aa.T                         [N,49]
    v_t = relu(vis @ Wv.T + bv)              [N,49,512]
    v_s = v_t @ Av.T                         [N,49,49]
    f   = tanh(a_s[:,:,None] + v_s) @ Af.T   [N,49]
    att = softmax_s(f);  out = att @ vis     [N,512]

Implementation notes:
  * v_t / v_s run as fp8e4 DoubleRow matmuls (0.5 cyc/row, K=256/instr).
    Scales: visT holds 4*vis, wv8 holds 8*Wv -> psum = 32*pre-act;
    vt8 = relu(psum + 32*bv) = 32*v_t; av8 = 8*Av -> v_s psum = 256*v_s;
    a_s enters the same psum scaled by 256 (host-scaled Aa); tanh uses
    scale=1/256.
  * Work is chunked by 120/120/80 "n" rows (psum partition limit) with
    row-tiles of 120/120/112 and 490-column vblocks (49*10, n-aligned).
  * out = att @ vis runs on the PE against the *untransposed* vis tiles:
    per row-tile a masked attention matrix Att[row, n'] (e values scattered
    to each row's n-column) is built from a PE column-transpose of the exp
    row + a small mask multiply; one accumulating matmul per tile.
  * Softmax is unnormalized; 1/Z folds into the final psum->sbuf copy.
  * Elementwise work (psum copies / relu) rotates across Act, DVE and
    GpSimd(Pool) engines to keep all three below the PE/DMA roofline.
"""

import numpy as np

try:
    import concourse.bass as bass
except ImportError:
    import sys as _sys
    for _p in ("/opt/trn_rl_repo", "/root/.axon_site/_ro/trn_rl_repo"):
        if _p not in _sys.path:
            _sys.path.insert(0, _p)
    import concourse.bass as bass
import concourse.mybir as mybir
import concourse.tile as tile
from concourse import bacc

F32 = mybir.dt.float32
F32R = mybir.dt.float32r
BF16 = mybir.dt.bfloat16
FP8 = mybir.dt.float8e4
AX = mybir.AxisListType
ALU = mybir.AluOpType
AF = mybir.ActivationFunctionType
DR = mybir.MatmulPerfMode.DoubleRow

NCORES = 8
B, T, S, D, E, A = 256, 10, 49, 512, 512, 128
N_N = (B // NCORES) * T          # 320 rows per core
ROWS = N_N * S                   # 15680 visual rows per core
VB = 490                         # vblock columns (10 n's)
HALF = 2940                      # visT half size (6 vblocks)
# (n0, n_count, tile_rows) per chunk; rows = n_count*49 divisible by both
# tile_rows and 490.
CHUNKS = [(0, 120, 120), (120, 120, 120), (240, 80, 112)]

# NOTE: GPSIMD (Pool) cannot access PSUM, so psum-reading ops rotate
# over Act/DVE only; Pool gets sbuf->sbuf work (Z-reduce, Att zeroing).
ACT, DVE, POOL = 0, 1, 2
COPY_PAT = [ACT, DVE]
RELU_PAT = [DVE, ACT]
ATT_PAT = [DVE]


def _tiles():
    """Global tile table: (chunk, t, row0_global, tile_rows)."""
    out = []
    for c, (n0c, ncn, tr) in enumerate(CHUNKS):
        rowsc = ncn * S
        for t in range(rowsc // tr):
            out.append((c, t, n0c * S + t * tr, tr))
    return out


TILES = _tiles()
NTILES = len(TILES)              # 133


def build_module():
    nc = bacc.Bacc("TRN2", debug=False)

    aud_d = nc.dram_tensor("audio", [N_N, A], F32R, kind="ExternalInput").ap()
    vis_d = nc.dram_tensor("visual", [ROWS, D], F32R, kind="ExternalInput").ap()
    wat_d = nc.dram_tensor("WaT", [128, E], F32R, kind="ExternalInput").ap()
    aat_d = nc.dram_tensor("AaT256", [128, 4, 64], F32R, kind="ExternalInput").ap()
    wv8_d = nc.dram_tensor("Wv8", [128, 2, 2, 4, 128], F32, kind="ExternalInput").ap()
    av8_d = nc.dram_tensor("Av8", [128, 2, 2, 64], F32, kind="ExternalInput").ap()
    aft_d = nc.dram_tensor("AfT", [S, 1], F32, kind="ExternalInput").ap()
    ones_d = nc.dram_tensor("ones", [1, 64], F32, kind="ExternalInput").ap()
    ba_d = nc.dram_tensor("ba_l", [128, 4], F32, kind="ExternalInput").ap()
    bv_d = nc.dram_tensor("bv32_l", [128, 4], F32, kind="ExternalInput").ap()
    idn_d = nc.dram_tensor("ident", [128, 128], F32R, kind="ExternalInput").ap()
    msk_d = nc.dram_tensor("masks", [128, NTILES, 4], F32, kind="ExternalInput").ap()
    zat_d = nc.dram_tensor("zeros128", [128, 128], F32R, kind="ExternalInput").ap()
    idf_d = nc.dram_tensor("identf1", [1, 1], F32, kind="ExternalInput").ap()
    out_d = nc.dram_tensor("out", [N_N, D], F32, kind="ExternalOutput").ap()

    with tile.TileContext(nc) as tc, \
         tc.tile_pool(name="consts", bufs=1) as cp, \
         tc.tile_pool(name="slab", bufs=3) as slp, \
         tc.tile_pool(name="visT", bufs=2) as vtp, \
         tc.tile_pool(name="vt8", bufs=2) as v8p, \
         tc.tile_pool(name="th", bufs=2) as thp, \
         tc.tile_pool(name="att", bufs=8) as atp, \
         tc.tile_pool(name="erow", bufs=2) as erp, \
         tc.tile_pool(name="asr", bufs=2) as asp, \
         tc.tile_pool(name="ecs", bufs=6) as ecp, \
         tc.tile_pool(name="outsb", bufs=2) as obp, \
         tc.tile_pool(name="small", bufs=2) as smp, \
         tc.tile_pool(name="dram", bufs=1, space="DRAM") as dp, \
         tc.tile_pool(name="ps_tr", bufs=2, space="PSUM") as ptr, \
         tc.tile_pool(name="ps_mm", bufs=3, space="PSUM") as pmm, \
         tc.tile_pool(name="ps_vs", bufs=2, space="PSUM") as pvs, \
         tc.tile_pool(name="ps_out", bufs=1, space="PSUM") as pou:

        # ------- small constants needed by the audio prologue (early) -------
        idn = cp.tile([128, 128], F32R, tag="idn")
        nc.sync.dma_start(idn[:], idn_d)
        idf = cp.tile([1, 1], F32, tag="idf")
        nc.sync.dma_start(idf[:], idf_d)
        wat = cp.tile([128, E], F32R, tag="wat")
        nc.sync.dma_start(wat[:], wat_d)
        aat = cp.tile([128, 4, 64], F32R, tag="aat")
        nc.sync.dma_start(aat[:], aat_d)
        ba = cp.tile([128, 4], F32, tag="ba")
        nc.sync.dma_start(ba[:], ba_d)
        bv32 = cp.tile([128, 4], F32, tag="bv32")
        nc.sync.dma_start(bv32[:], bv_d)
        aftf = cp.tile([S, 1], F32, tag="aftf")
        nc.sync.dma_start(aftf[:], aft_d)
        onesf = cp.tile([1, 64], F32, tag="onesf")
        nc.sync.dma_start(onesf[:], ones_d)

        audT = cp.tile([128, N_N], F32R, tag="audT")
        atT = cp.tile([128, 4, N_N], F32R, tag="atT")
        zrow = cp.tile([1, 128], F32, tag="zrow")
        as_dram = dp.tile([1, ROWS], BF16, tag="asd")

        # engine-rotating elementwise helpers ------------------------------
        def cast_copy(eng, dst, src, scale=None):
            if eng == ACT:
                if scale is None:
                    nc.scalar.activation(dst, src, AF.Copy)
                else:
                    nc.scalar.activation(dst, src, AF.Copy, scale=scale)
            else:
                e = nc.vector if eng == DVE else nc.gpsimd
                if scale is None:
                    e.tensor_copy(dst, src)
                else:
                    e.tensor_scalar(dst, src, float(scale), None, ALU.mult)

        def relu_op(eng, dst, src, bias_ap):
            if eng == ACT:
                nc.scalar.activation(dst, src, AF.Relu, bias=bias_ap)
            else:
                e = nc.vector if eng == DVE else nc.gpsimd
                e.tensor_scalar(dst, src, bias_ap, 0.0, ALU.add, ALU.max)

        copy_i = [0]
        relu_i = [0]
        att_i = [0]

        def rot(pat, i):
            e = pat[i[0] % len(pat)]
            i[0] += 1
            return e

        slabs = {}

        def prefetch_slab(cc, si):
            trc = CHUNKS[cc][2]
            sl = slp.tile([trc, 7, D], F32R, tag="slab",
                          name=f"slab{cc}_{si}")
            slabs[(cc, si)] = sl
            r0 = CHUNKS[cc][0] * S + si * 7 * trc
            src = vis_d[r0:r0 + 7 * trc, :]
            nc.sync.dma_start(sl[:], src.rearrange("(j p) d -> p j d", j=7))

        # ---------------- audio prologue ----------------
        an_tiles = []
        for it in range(3):
            n0 = it * 128
            nr = min(128, N_N - n0)
            an = smp.tile([128, A], F32R, tag="an")
            nc.sync.dma_start(an[:nr, :], aud_d[n0:n0 + nr, :])
            an_tiles.append((an, n0, nr))
        # start the first visual slabs early, right behind the audio DMA
        prefetch_slab(0, 0)
        prefetch_slab(0, 1)
        for an, n0, nr in an_tiles:
            ps = ptr.tile([128, 4, 128], F32, tag="tr")
            nc.tensor.transpose(ps[:, 0, :nr].bitcast(F32R), an[:nr, :],
                                idn[:nr, :nr])
            nc.scalar.copy(audT[:, n0:n0 + nr], ps[:, 0, :nr])

        for eo in range(4):
            ps = pmm.tile([128, VB], F32, tag="mm")
            nc.tensor.matmul(ps[:, :N_N], wat[:, eo * 128:(eo + 1) * 128],
                             audT[:], start=True, stop=True)
            nc.scalar.activation(atT[:, eo, :], ps[:, :N_N], AF.Relu,
                                 bias=ba[:, eo:eo + 1])

        for it in range(3):
            n0 = it * 128
            nr = min(128, N_N - n0)
            ps = pmm.tile([128, VB], F32, tag="mm")
            for eo in range(4):
                nc.tensor.matmul(ps[:nr, :64], atT[:, eo, n0:n0 + nr],
                                 aat[:, eo, :], start=(eo == 0), stop=(eo == 3))
            asn = smp.tile([128, S], BF16, tag="asn")
            nc.scalar.copy(asn[:nr, :], ps[:nr, :S])
            dst = as_dram[0:1, n0 * S:(n0 + nr) * S]
            nc.gpsimd.dma_start(dst.rearrange("one (n s) -> (one n) s", s=S),
                                asn[:nr, :])

        # ------- fat constants (not needed in the first ~8us) -------
        wv8f = cp.tile([128, 2, 2, 4, 128], F32, tag="wv8f")
        nc.sync.dma_start(wv8f[:], wv8_d)
        av8f = cp.tile([128, 2, 2, 64], F32, tag="av8f")
        nc.sync.dma_start(av8f[:], av8_d)
        masks = cp.tile([128, NTILES, 4], F32, tag="masks")
        nc.sync.dma_start(masks[:], msk_d)
        zat = cp.tile([128, 128], F32R, tag="zat")
        nc.sync.dma_start(zat[:], zat_d)

        wv8 = cp.tile([128, 2, 2, 4, 128], FP8, tag="wv8")
        nc.scalar.activation(wv8[:], wv8f[:], AF.Copy)
        av8 = cp.tile([128, 2, 2, 64], FP8, tag="av8")
        nc.scalar.activation(av8[:], av8f[:], AF.Copy)
        aftb = cp.tile([S, 1], BF16, tag="aftb")
        nc.scalar.activation(aftb[:], aftf[:], AF.Copy)
        onesb = cp.tile([1, 64], BF16, tag="onesb")
        nc.scalar.activation(onesb[:], onesf[:], AF.Copy)

        att_bufs = []
        for i in range(8):
            ab = atp.tile([128, 128], F32R, tag="att", name=f"att{i}")
            nc.sync.dma_start(ab[:], zat_d)
            att_bufs.append(ab)
        att_cols = [None] * 8           # (jc0, m) of last use per buf

        # ---------------- main loop ----------------
        gt0 = 0  # global tile index of current chunk's tile 0
        for c, (n0c, ncn, trh) in enumerate(CHUNKS):
            rowsc = ncn * S
            row0c = n0c * S
            ntile = rowsc // trh
            nvb = rowsc // VB

            asr = asp.tile([1, 6 * VB * 2], BF16, tag="asr")
            nc.gpsimd.dma_start(asr[0:1, 0:rowsc],
                                as_dram[0:1, row0c:row0c + rowsc])
            pso = pou.tile([128, D], F32, tag="out")
            erow = erp.tile([1, 6 * VB * 2], F32, tag="erow")
            halves = [vtp.tile([128, 4, HALF], FP8, tag="visT", name=f"visT{c}_{h}")
                      for h in range(2)]

            vt8_map = {}
            th_map = {}
            psv_map = {}

            # --- software-pipelined vblock stages, one vblock apart -----
            def stage_A(vb):
                """v_t DoubleRow matmuls + relu stores."""
                h = vb // 6
                cs = vb * VB - h * HALF
                vt8 = v8p.tile([128, 2, 2, VB], FP8, tag="vt8")
                vt8_map[vb] = vt8
                for eo in range(4):
                    psm = pmm.tile([128, VB], F32, tag="mm")
                    for h2 in range(2):
                        nc.tensor.matmul(psm[:], wv8[:, h2, :, eo, :],
                                         halves[h][:, 2 * h2:2 * h2 + 2, cs:cs + VB],
                                         start=(h2 == 0), stop=(h2 == 1),
                                         perf_mode=DR)
                    relu_op(rot(RELU_PAT, relu_i), vt8[:, eo // 2, eo % 2, :],
                            psm[:], bv32[:, eo:eo + 1])

            def stage_B(vb):
                """v_s DoubleRow + a_s add + tanh."""
                vt8 = vt8_map.pop(vb)
                psv = pvs.tile([128, 497], F32, tag="vs")
                psv_map[vb] = psv
                for h2 in range(2):
                    nc.tensor.matmul(psv[0:64, 0:VB], av8[:, h2, :, :],
                                     vt8[:, h2, :, :],
                                     start=(h2 == 0), stop=False, perf_mode=DR)
                nc.tensor.matmul(psv[0:64, 0:VB], onesb[:],
                                 asr[0:1, vb * VB:(vb + 1) * VB],
                                 start=False, stop=True)
                th = thp.tile([S, VB], BF16, tag="th")
                th_map[vb] = th
                nc.scalar.activation(th[:], psv[:S, 0:VB], AF.Tanh,
                                     scale=1.0 / 256.0)

            def stage_C(vb):
                """f row (into psv row 0) + exp + Z partial sums."""
                psv = psv_map[vb]
                th = th_map.pop(vb)
                nc.tensor.matmul(psv[0:1, 0:VB], aftb[:], th[:],
                                 start=True, stop=True)
                nc.scalar.activation(erow[0:1, vb * VB:(vb + 1) * VB],
                                     psv[0:1, 0:VB], AF.Exp)
                nc.vector.reduce_sum(
                    zrow[0:1, vb * 10:(vb + 1) * 10],
                    erow[0:1, vb * VB:(vb + 1) * VB]
                    .rearrange("p (n s) -> p n s", s=S), axis=AX.X)

            def emit_eta(t):
                """e-col transpose (PE) -> sbuf stage (DVE) -> Att (Pool)."""
                gt = gt0 + t
                tr = trh
                # slot lives in the psum tile that held exp's f-row
                vcov = ((t + 1) * tr - 1) // VB
                ecol = psv_map[vcov][:, 490 + t % 6:491 + t % 6]
                nc.tensor.transpose(ecol[0:tr, :],
                                    erow[0:1, t * tr:t * tr + tr],
                                    idf[0:1, 0:1])
                ecs = ecp.tile([128, 1], F32, tag="ecs")
                nc.vector.tensor_copy(ecs[0:tr, :], ecol[0:tr, :])
                row0 = TILES[gt][2]
                jc0 = row0 // S - n0c
                m = (row0 + tr - 1) // S - row0 // S + 1
                ab = att_bufs[gt % 8]
                stale = att_cols[gt % 8]
                if stale is not None:
                    s0, s1 = stale
                    nc.gpsimd.tensor_copy(ab[:, s0:s1], zat[:, 0:s1 - s0])
                att_cols[gt % 8] = (jc0, jc0 + m)
                nc.gpsimd.tensor_tensor(ab[0:tr, jc0:jc0 + m],
                                        masks[0:tr, gt, 0:m],
                                        ecs[0:tr, 0:1].broadcast_to([tr, m]),
                                        ALU.mult)

            def emit_out(t):
                slab = slabs[(c, t // 7)]
                tr = trh
                nc.tensor.matmul(pso[:], att_bufs[(gt0 + t) % 8][0:tr, :],
                                 slab[0:tr, t % 7, :],
                                 start=(t == 0), stop=(t == ntile - 1))

            state = {"k": 0, "eta": 0, "out": 0}

            def emit_eta_guarded():
                # att buf k%8 is recycled at eta(k+8): out(k) must be
                # emitted before eta(k+8) clobbers its values.
                while state["out"] <= state["eta"] - 8:
                    emit_out(state["out"])
                    state["out"] += 1
                emit_eta(state["eta"])
                state["eta"] += 1

            def iteration():
                k = state["k"]
                # etas read exp(k-3), emitted one iteration ago; outs lag
                # one more iteration behind the etas.
                while (state["eta"] + 1) * trh <= (k - 2) * VB:
                    emit_eta_guarded()
                while state["out"] < state["eta"] and \
                        (state["out"] + 1) * trh <= (k - 3) * VB:
                    emit_out(state["out"])
                    state["out"] += 1
                if k < nvb:
                    stage_A(k)
                if 0 <= k - 1 < nvb:
                    stage_B(k - 1)
                if 0 <= k - 2 < nvb:
                    stage_C(k - 2)
                state["k"] += 1

            def emit_tile(t):
                if t % 7 == 0:
                    key = (c, t // 7)
                    if key not in slabs:
                        sl = slp.tile([trh, 7, D], F32R, tag="slab",
                                      name=f"slab{c}_{t // 7}")
                        slabs[key] = sl
                        r0 = row0c + t * trh
                        src = vis_d[r0:r0 + 7 * trh, :]
                        nc.sync.dma_start(
                            sl[:], src.rearrange("(j p) d -> p j d", j=7))
                ps = ptr.tile([128, 4, 128], F32, tag="tr")
                for do in range(4):
                    nc.tensor.transpose(ps[:, do, :trh].bitcast(F32R),
                                        slabs[(c, t // 7)][0:trh, t % 7,
                                                           do * 128:(do + 1) * 128],
                                        idn[:trh, :trh])
                c0 = t * trh
                c1 = c0 + trh
                for h in range(2):
                    lo = max(c0, h * HALF)
                    hi = min(c1, HALF + h * (rowsc - HALF))
                    if lo < hi:
                        cast_copy(rot(COPY_PAT, copy_i),
                                  halves[h][:, :, lo - h * HALF:hi - h * HALF],
                                  ps[:, :, lo - c0:hi - c0], scale=4.0)

            def pump(t):
                # stage A(k) consumes visT cols [k*VB,(k+1)*VB); one extra
                # tile of lag lets the copies clear the Act/DVE queues
                while (state["k"] + 1) * VB <= t * trh:
                    iteration()

            for t in range(ntile):
                emit_tile(t)
                pump(t)
            while state["k"] < nvb + 2:
                iteration()
            while state["eta"] < ntile:
                emit_eta_guarded()
            while state["out"] < ntile:
                emit_out(state["out"])
                state["out"] += 1

            # chunk epilogue: rinv + scaled copy + store
            psl = psv_map[nvb - 1]
            nc.tensor.transpose(psl[0:ncn, 496:497],
                                zrow[0:1, 0:ncn], idf[0:1, 0:1])
            zcol = smp.tile([128, 1], F32, tag="zcol")
            nc.vector.tensor_copy(zcol[0:ncn, :], psl[0:ncn, 496:497])
            rin = smp.tile([128, 1], F32, tag="rin")
            nc.vector.reciprocal(rin[0:ncn, :], zcol[0:ncn, :])
            ob = obp.tile([128, D], F32, tag="ob")
            nc.scalar.activation(ob[0:ncn, :], pso[0:ncn, :], AF.Copy,
                                 scale=rin[0:ncn, 0:1])
            nc.gpsimd.dma_start(out_d[n0c:n0c + ncn, :], ob[0:ncn, :])
            gt0 += ntile

    nc.finalize()
    return nc


def prep_consts(Wa, ba_, Wv, bv_, Aa, Av, Af):
    f = np.float32
    c = {}
    c["WaT"] = np.ascontiguousarray(Wa.T).astype(f)
    aat = np.zeros((128, 4, 64), f)
    # aat[p, eo, s] = 256*Aa[s, eo*128+p]
    aat[:, :, :S] = (256.0 * Aa.T).reshape(4, 128, S).transpose(1, 0, 2)
    c["AaT256"] = aat
    # wv8[p, h, i, eo, m] = 8*Wv[eo*128+m, h*256+i*128+p]
    w = 8.0 * Wv.astype(f)                   # [e, d]
    w = w.T.reshape(2, 2, 128, 4, 128)       # [h, i, p, eo, m]
    c["Wv8"] = np.ascontiguousarray(w.transpose(2, 0, 1, 3, 4))
    a = np.zeros((64, 512), f)               # pad f-dim to 64 (DR needs M=64/128)
    a[:S] = 8.0 * Av.astype(f)
    a = a.T.reshape(2, 2, 128, 64)           # [h, i, p, f]
    c["Av8"] = np.ascontiguousarray(a.transpose(2, 0, 1, 3))
    c["AfT"] = np.ascontiguousarray(Af.reshape(1, S).T).astype(f)
    c["ones"] = np.ones((1, 64), f)
    c["ba_l"] = np.ascontiguousarray(ba_.reshape(4, 128).T).astype(f)
    c["bv32_l"] = np.ascontiguousarray(32.0 * bv_.reshape(4, 128).T).astype(f)
    c["ident"] = np.eye(128, dtype=f)
    c["identf1"] = np.ones((1, 1), f)
    c["zeros128"] = np.zeros((128, 128), f)
    masks = np.zeros((128, NTILES, 4), f)
    for gt, (cc, t, row0, tr) in enumerate(TILES):
        n0 = row0 // S
        for p in range(tr):
            j = (row0 + p) // S - n0
            masks[p, gt, j] = 1.0
    c["masks"] = masks
    return c


_CACHE = {}


def kernel(audio, visual, Wa, ba, Wv, bv, Aa, Av, Af):
    from concourse.bass_utils import run_bass_kernel_spmd

    audio = np.asarray(audio, np.float32)
    visual = np.asarray(visual, np.float32)

    if "nc" not in _CACHE:
        _CACHE["nc"] = build_module()
    nc = _CACHE["nc"]

    consts = prep_consts(np.asarray(Wa, np.float32), np.asarray(ba, np.float32),
                         np.asarray(Wv, np.float32), np.asarray(bv, np.float32),
                         np.asarray(Aa, np.float32), np.asarray(Av, np.float32),
                         np.asarray(Af, np.float32))
    bs = B // NCORES
    in_maps = []
    for cid in range(NCORES):
        m = dict(consts)
        m["audio"] = np.ascontiguousarray(
            audio[cid * bs:(cid + 1) * bs].reshape(N_N, A))
        m["visual"] = np.ascontiguousarray(
            visual[cid * bs:(cid + 1) * bs].reshape(ROWS, D))
        in_maps.append(m)

    res = run_bass_kernel_spmd(nc, in_maps, core_ids=list(range(NCORES)))
    _CACHE["last_res"] = res
    out = np.concatenate(
        [r["out"].reshape(bs, T, D) for r in res.results], axis=0)
    return out.astype(np.float32)
